# revision 2
# baseline (speedup 1.0000x reference)
"""Multi-head attention (B=2, N=4096, D=512, H=8) on 8 trn2 NeuronCores.

Sharding: core c handles batch b = c//4 and head-pair p = c%4 (heads 2p,
2p+1).  Each core projects its batch's Q/K/V against its pair's weight
columns, computes transposed attention scores sT = K_h @ Q_h^T, applies
exp((1/sqrt(dk))*sT) split between the ACT engine (table exp) and the
DVE (Schraudolph int16 bit-trick: e = bitcast_fp16(int16(A*s + B)); the
method's constant ratio offset cancels in softmax), multiplies by an
augmented V (extra ones column, M=65) so the softmax denominators fall
out of the same matmul, and applies its rows of Wo with both heads
row-tiled concurrently in the PE array.  Normalization by the softmax
denominator commutes with the output projection, so it is applied on
the host during the cross-core reduction.

Device layouts (host pre-arranges):
  xt{q,k,v}: X^T            [512, N]  (D on partitions when tiled)
  w{q,k,v}:  [128, 4, 128]  w[p, dc, c] = W[dc*128+p, off+c]
  b{q,k,v}:  [128, 1]       pair slice of bias
  wo:        [128, 4, 128]  wo[p, mt, c] = Wo[off+p, mt*128+c]
Outputs per core:
  y0, y1: [512, N]  y_h[dout, q] = (O_un_h @ Wo_h)^T  (unnormalized)
  den:    [2, N]    softmax denominators per head
Final host step: out[b] = (sum_{p,h} y_h / den_h).T + bo
"""

import numpy as np

_B, _N, _D, _H, _DK = 2, 4096, 512, 8, 64
_NCORES = 8

# Schraudolph fp16 exp constants: e^x ~= bitcast_fp16(int16(A*x + B)).
# A folds the 1/sqrt(dk)=0.125 score scale; B is re-centered so the mean
# ratio to true exp is ~1.0 (floor conversion baseline mean is ~1.0407,
# i.e. log2(1.0407)*1024 ~= 59.5).
_LN2 = float(np.log(2.0))
_A_DVE = 0.125 * 1024.0 / _LN2
_B_DVE = 15.0 * 1024.0 - 59.5

_nc_cache = {}


def _build(n=_N, zero_bias=False):
    import concourse.mybir as mybir
    import concourse.tile as tile
    from concourse import bacc
    from concourse.masks import make_identity

    f32 = mybir.dt.float32
    i16 = mybir.dt.int16
    bf16 = mybir.dt.float16
    Exp = mybir.ActivationFunctionType.Exp
    D = _D
    NKC = n // 128  # k chunks of 128 (columns of sT)
    NQC = n // 512  # q chunks of 512
    blocks = []
    i = 0
    while i < NKC:
        blen = min(3, NKC - i)
        blocks.append((i, blen))
        i += blen

    # ACT/DVE exp split point within a block of blen*512 columns.
    # ACT rate ~1.2 GHz, DVE ~0.96 GHz (both 1 elem/cycle/lane here), so
    # ACT takes ~54%.
    def act_cols(blen):
        return (int(blen * 512 * 0.54) // 32) * 32

    nc = bacc.Bacc(
        "TRN2", target_bir_lowering=False, debug=False, num_devices=_NCORES
    )

    xt = {
        t: nc.dram_tensor(f"xt{t}", [D, n], bf16, kind="ExternalInput").ap()
        for t in "qkv"
    }
    w = {
        t: nc.dram_tensor(f"w{t}", [128, 4, 128], bf16, kind="ExternalInput").ap()
        for t in "qkv"
    }
    bvec = {
        t: nc.dram_tensor(f"b{t}", [128, 1], f32, kind="ExternalInput").ap()
        for t in "qkv"
    }
    wo = nc.dram_tensor("wo", [128, 4, 128], bf16, kind="ExternalInput").ap()
    y_out = [
        nc.dram_tensor(f"y{h}", [D, n], bf16, kind="ExternalOutput").ap()
        for h in range(2)
    ]
    den_out = nc.dram_tensor("den", [2, n], f32, kind="ExternalOutput").ap()

    with tile.TileContext(nc) as tc:
        with (
            tc.tile_pool(name="consts", bufs=1) as consts,
            tc.tile_pool(name="xtp", bufs=8) as xtp,
            tc.tile_pool(name="persist", bufs=1) as persist,
            tc.tile_pool(name="ep", bufs=4) as ep,
            tc.tile_pool(name="psA", bufs=2, space="PSUM") as psA,
            tc.tile_pool(name="psB", bufs=2, space="PSUM") as psB,
        ):
            ident = consts.tile([128, 128], bf16, name="ident")
            make_identity(nc, ident)
            wsb, bsb = {}, {}
            for t in "qkv":
                wsb[t] = consts.tile([128, 4, 128], bf16, name=f"w{t}sb", tag=f"w{t}sb")
                nc.sync.dma_start(out=wsb[t], in_=w[t])
                bsb[t] = consts.tile([128, 1], f32, name=f"b{t}sb", tag=f"b{t}sb")
                nc.sync.dma_start(out=bsb[t], in_=bvec[t])
            wosb = consts.tile([128, 4, 128], bf16, name="wosb", tag="wosb")
            nc.sync.dma_start(out=wosb, in_=wo)

            NNC = n // 512
            qt_t = [
                persist.tile([128, 512], bf16, name=f"qt{i}", tag=f"qt{i}")
                for i in range(NNC)
            ]
            kt_t = [
                persist.tile([128, 512], bf16, name=f"kt{i}", tag=f"kt{i}")
                for i in range(NNC)
            ]
            vt_t = [
                persist.tile([128, 512], bf16, name=f"vt{i}", tag=f"vt{i}")
                for i in range(NNC)
            ]
            # augmented V chunks: 64 head dims + ones column (col 64)
            vch = [
                [
                    persist.tile(
                        [128, 65], bf16, name=f"vch{h}_{c}", tag=f"vch{h}_{c}"
                    )
                    for c in range(NKC)
                ]
                for h in range(2)
            ]
            # per-qc attention output, both heads packed: rows 64h..64h+63
            ot = [
                persist.tile([128, 512], bf16, name=f"ot{qc}", tag=f"ot{qc}")
                for qc in range(NQC)
            ]
            den_sb = [
                persist.tile([1, n], f32, name=f"den{h}", tag=f"den{h}")
                for h in range(2)
            ]
            for h in range(2):
                for c in range(NKC):
                    nc.vector.memset(vch[h][c][:, 64:65], 1.0)

            # ---- phase 1: projections  t^T = W_p^T @ X^T + b ----
            # nk-outer so early chunks of q/k/v unlock attention ASAP
            dest = {"q": qt_t, "k": kt_t, "v": vt_t}
            for nk in range(NQC):
                for t in "qkv":
                    ppsum = psA.tile([128, 512], f32, name=f"pp_{t}{nk}", tag="s")
                    for dc in range(4):
                        xtile = xtp.tile(
                            [128, 512], bf16, name=f"x_{t}{nk}{dc}", tag="xt"
                        )
                        nc.sync.dma_start(
                            out=xtile,
                            in_=xt[t][dc * 128 : (dc + 1) * 128, nk * 512 : (nk + 1) * 512],
                        )
                        nc.tensor.matmul(
                            ppsum,
                            wsb[t][:, dc, :],
                            xtile,
                            start=(dc == 0),
                            stop=(dc == 3),
                        )
                    if zero_bias:
                        nc.scalar.activation(
                            out=dest[t][nk],
                            in_=ppsum,
                            func=mybir.ActivationFunctionType.Copy,
                        )
                    else:
                        nc.vector.tensor_scalar_add(
                            out=dest[t][nk], in0=ppsum, scalar1=bsb[t]
                        )
                # v^T chunks -> per-head augmented layout
                for c in range(nk * 4, nk * 4 + 4):
                    pt = psA.tile([128, 512], bf16, name=f"pt{c}", tag="s")
                    nc.tensor.transpose(
                        pt[:, 0:128],
                        vt_t[c // 4][:, (c % 4) * 128 : (c % 4 + 1) * 128],
                        ident,
                    )
                    for h in range(2):
                        nc.vector.tensor_copy(
                            out=vch[h][c][:, 0:64], in_=pt[:, h * 64 : (h + 1) * 64]
                        )

            # ---- phase 2: attention, both heads as concurrent streams ----
            for qc in range(NQC):
                qs = slice(qc * 512, (qc + 1) * 512)
                o_ps = {
                    h: psB.tile([128, 512], f32, name=f"o_{h}_{qc}", tag="oy")
                    for h in range(2)
                }
                for k0, blen in blocks:
                    for h in range(2):
                        hp = slice(h * 64, (h + 1) * 64)
                        s_ps = psA.tile(
                            [128, blen * 512], f32, name=f"s_{h}_{qc}_{k0}", tag="s"
                        )
                        for j in range(blen):
                            kc = k0 + j
                            nc.tensor.matmul(
                                s_ps[:, j * 512 : (j + 1) * 512],
                                kt_t[kc // 4][hp, (kc % 4) * 128 : (kc % 4 + 1) * 128],
                                qt_t[qc][hp, :],
                                start=True,
                                stop=True,
                                skip_group_check=True,
                            )
                        e_sb = ep.tile(
                            [128, blen * 512], bf16, name=f"e_{h}_{qc}_{k0}", tag="e"
                        )
                        u = act_cols(blen)
                        nc.scalar.activation(
                            e_sb[:, 0:u], s_ps[:, 0:u], Exp, scale=0.125
                        )
                        nc.vector.tensor_scalar(
                            out=e_sb[:, u:].bitcast(i16),
                            in0=s_ps[:, u:],
                            scalar1=_A_DVE,
                            scalar2=_B_DVE,
                            op0=mybir.AluOpType.mult,
                            op1=mybir.AluOpType.add,
                        )
                        for j in range(blen):
                            kc = k0 + j
                            nc.tensor.matmul(
                                o_ps[h][0:65, :],
                                vch[h][kc],
                                e_sb[:, j * 512 : (j + 1) * 512],
                                start=(kc == 0),
                                stop=(kc == NKC - 1),
                                skip_group_check=True,
                            )
                for h in range(2):
                    nc.vector.tensor_copy(
                        out=ot[qc][h * 64 : (h + 1) * 64, :], in_=o_ps[h][0:64, :]
                    )
                    nc.vector.tensor_copy(
                        out=den_sb[h][0:1, qs], in_=o_ps[h][64:65, :]
                    )

            # ---- phase 3: out-projection, heads row-tiled concurrently ----
            for qc in range(NQC):
                qs = slice(qc * 512, (qc + 1) * 512)
                for mt in range(4):
                    for h in range(2):
                        hp = slice(h * 64, (h + 1) * 64)
                        pool, tag = (psA, "s") if h == 0 else (psB, "oy")
                        y_ps = pool.tile(
                            [128, 512], f32, name=f"y_{h}_{qc}_{mt}", tag=tag
                        )
                        nc.tensor.matmul(
                            y_ps,
                            wosb[hp, mt, :],
                            ot[qc][hp, :],
                            start=True,
                            stop=True,
                            skip_group_check=True,
                        )
                        y_sb = xtp.tile(
                            [128, 512], bf16, name=f"ysb_{h}_{qc}_{mt}", tag="ysb"
                        )
                        if h == 0:
                            nc.vector.tensor_copy(out=y_sb, in_=y_ps)
                        else:
                            nc.scalar.copy(out=y_sb, in_=y_ps)
                        nc.sync.dma_start(
                            out=y_out[h][mt * 128 : (mt + 1) * 128, qs], in_=y_sb
                        )
            for h in range(2):
                nc.sync.dma_start(out=den_out[h : h + 1, :], in_=den_sb[h][0:1, :])
    nc.finalize()
    return nc


def get_nc(n=_N, zero_bias=False):
    key = (n, zero_bias)
    if key not in _nc_cache:
        _nc_cache[key] = _build(n, zero_bias)
    return _nc_cache[key]


def make_in_maps(Q, K, V, Wq, bq, Wk, bk, Wv, bv, Wo, bo, n=_N):
    """Per-core input dicts (host-side sharding / layout prep)."""
    bf = np.float16
    xts = {}
    for b in range(_B):
        xts[b] = {
            "xtq": np.ascontiguousarray(Q[b][:n].T.astype(bf)),
            "xtk": np.ascontiguousarray(K[b][:n].T.astype(bf)),
            "xtv": np.ascontiguousarray(V[b][:n].T.astype(bf)),
        }
    in_maps = []
    for c in range(_NCORES):
        b, p = divmod(c, 4)
        off = p * 128
        m = dict(xts[b])
        for t, W, bias in (("q", Wq, bq), ("k", Wk, bk), ("v", Wv, bv)):
            m[f"w{t}"] = np.ascontiguousarray(
                W[:, off : off + 128].reshape(4, 128, 128).transpose(1, 0, 2).astype(bf)
            )
            m[f"b{t}"] = np.ascontiguousarray(bias[off : off + 128].reshape(128, 1))
        m["wo"] = np.ascontiguousarray(
            Wo[off : off + 128].reshape(128, 4, 128).astype(bf)
        )
        in_maps.append(m)
    return in_maps


def assemble(results, bo, n=_N):
    """Cross-core reduction: normalize by softmax denominators, sum heads,
    add output bias, restore [B, N, D] layout."""
    out = np.empty((_B, n, _D), np.float32)
    for b in range(_B):
        acc = np.zeros((_D, n), np.float32)
        for p in range(4):
            r = results[4 * b + p]
            for h in range(2):
                acc += r[f"y{h}"].astype(np.float32) / r["den"][h][None, :]
        out[b] = acc.T + bo
    return out


def kernel(Q, K, V, Wq, bq, Wk, bk, Wv, bv, Wo, bo):
    from concourse import bass_utils

    args = [np.asarray(a, np.float32) for a in (Q, K, V, Wq, bq, Wk, bk, Wv, bv, Wo, bo)]
    Q, K, V, Wq, bq, Wk, bk, Wv, bv, Wo, bo = args
    zb = not (np.any(bq) or np.any(bk) or np.any(bv))
    nc = get_nc(zero_bias=zb)
    in_maps = make_in_maps(Q, K, V, Wq, bq, Wk, bk, Wv, bv, Wo, bo)
    res = bass_utils.run_bass_kernel_spmd(
        nc, in_maps, core_ids=list(range(_NCORES))
    )
    return assemble(res.results, bo)


# revision 9
# speedup vs baseline: 1.2431x; 1.2431x over previous
"""Multi-head attention (B=2, N=4096, D=512, H=8) on 8 trn2 NeuronCores.

Sharding: core c handles batch b = c//4 and head-pair p = c%4 (heads 2p,
2p+1).  Each core projects its batch's Q/K/V against its pair's weight
columns, computes transposed attention scores sT = K_h @ Q_h^T, applies
exp((1/sqrt(dk))*sT) split between the ACT engine (table exp, ~75% of
columns) and the DVE (~25%: a Schraudolph int16 bit-trick pass plus a
custom 7-stage correction op that multiplies by a quadratic in the
mantissa, sigma ~0.2%), multiplies by an augmented V (extra ones column,
M=65) so the softmax denominators fall out of the same matmul, and
applies its rows of Wo with both heads row-tiled concurrently in the PE
array.  The attn@V matmuls are issued one k-block behind the scores
matmuls so the PE never head-of-line blocks on exp results (keeps the
HAM un-throttled).  Normalization by the softmax denominator commutes
with the output projection, so it is applied on the host during the
cross-core reduction.

Device layouts (host pre-arranges):
  xt{q,k,v}: X^T            [512, N]  (D on partitions when tiled)
  w{q,k,v}:  [128, 4, 128]  w[p, dc, c] = W[dc*128+p, off+c]
  b{q,k,v}:  [128, 1]       pair slice of bias
  wo:        [128, 4, 128]  wo[p, mt, c] = Wo[off+p, mt*128+c]
Outputs per core:
  y0, y1: [512, N]  y_h[dout, q] = (O_un_h @ Wo_h)^T  (unnormalized)
  den:    [2, N]    softmax denominators per head
Final host step: out[b] = (sum_{p,h} y_h / den_h).T + bo
"""

import numpy as np

_B, _N, _D, _H, _DK = 2, 4096, 512, 8, 64
_NCORES = 8

# Schraudolph fp16 exp constants: e0 = bitcast_fp16(int16(A*s + B)); the
# custom correction op then computes e = e0 * (q2*(u+a)^2 + c) with
# u = 1 + mantissa_frac(e0).  Constants calibrated bit-exactly against
# np.exp (ratio mean 1.0 +- 0.002).
_LN2 = float(np.log(2.0))
_A_DVE = 0.125 * 1024.0 / _LN2
_B_DVE = 15360.0
_CORR_A = -1.4763417585548537
_CORR_Q2 = 0.22711289921196798
_CORR_C = 0.9424678640725361

_nc_cache = {}
_exp_corr_op = None


def _get_exp_corr_op():
    """Register (once) the custom DVE op: out = ((u+C0)^2*C1 + C2) * Src0
    with u = (Src0 & Src1) | 1.0 — Src1 carries the fp32 mantissa mask
    0x007FFFFF as a [P,1] broadcast."""
    global _exp_corr_op
    if _exp_corr_op is not None:
        return _exp_corr_op
    from concourse import dve_ops
    from concourse.dve_spec import (
        AluOp,
        Bin,
        C0,
        C1,
        C2,
        One,
        Spec,
        Src0,
        Src1,
        lower,
        sq,
    )
    from concourse.dve_uop import DveOpSpec

    name = "EXP16_CORR_ANT"
    for op in dve_ops.OPS:
        if op.name == name:
            _exp_corr_op = op
            return op

    u = Bin(AluOp.BITWISE_OR, Bin(AluOp.BITWISE_AND, Src0, Src1), One)
    body = (sq(u + C0) * C1 + C2) * Src0

    def _ref(in0, in1, s0, s1, imm2):
        b = np.asarray(in0, np.float32).view(np.uint32)
        m = np.asarray(in1, np.float32).view(np.uint32)
        uu = ((b & m) | np.uint32(0x3F800000)).view(np.float32)
        return ((uu + s0) ** 2 * s1 + imm2) * in0

    spec = Spec(body=body, reference=_ref)
    sha = {
        ver: DveOpSpec(name=name, uops=lower(spec, ver=ver)).sha(ver)
        for ver in ("v3", "v4")
    }
    op = dve_ops.DveOp(name, spec, subdim=False, uops_sha=sha)
    idx = len(dve_ops.OPS)
    dve_ops.OPS.append(op)
    dve_ops.CUSTOM_DVE_SPECS[name] = spec
    dve_ops._SUB_OPCODE_FOR_NAME[name] = dve_ops._CUSTOM_DVE_ROW_BASE + idx
    _exp_corr_op = op
    return op


def _build(n=_N, zero_bias=False, dve_split=True):
    import concourse.mybir as mybir
    import concourse.tile as tile
    from concourse import bacc
    from concourse.masks import make_identity

    f32 = mybir.dt.float32
    i16 = mybir.dt.int16
    i32 = mybir.dt.int32
    bf16 = mybir.dt.float16
    Exp = mybir.ActivationFunctionType.Exp
    D = _D
    NKC = n // 128  # k chunks of 128 (columns of sT)
    NQC = n // 512  # q chunks of 512
    blocks = []
    i = 0
    while i < NKC:
        blen = min(3, NKC - i)
        blocks.append((i, blen))
        i += blen

    corr_op = _get_exp_corr_op() if dve_split else None

    # ACT/DVE exp split: ACT gets ~75% of each block's columns (DVE pays
    # two instruction passes per element).
    def act_cols(blen):
        if not dve_split:
            return blen * 512
        return (int(blen * 512 * 0.75) // 32) * 32

    nc = bacc.Bacc(
        "TRN2", target_bir_lowering=False, debug=False, num_devices=_NCORES
    )

    xt = {
        t: nc.dram_tensor(f"xt{t}", [D, n], bf16, kind="ExternalInput").ap()
        for t in "qkv"
    }
    w = {
        t: nc.dram_tensor(f"w{t}", [128, 4, 128], bf16, kind="ExternalInput").ap()
        for t in "qkv"
    }
    bvec = {
        t: nc.dram_tensor(f"b{t}", [128, 1], f32, kind="ExternalInput").ap()
        for t in "qkv"
    }
    wo = nc.dram_tensor("wo", [128, 4, 128], bf16, kind="ExternalInput").ap()
    y_out = [
        nc.dram_tensor(f"y{h}", [D, n], bf16, kind="ExternalOutput").ap()
        for h in range(2)
    ]
    den_out = nc.dram_tensor("den", [2, n], f32, kind="ExternalOutput").ap()

    with tile.TileContext(nc) as tc:
        with (
            tc.tile_pool(name="consts", bufs=1) as consts,
            tc.tile_pool(name="xtp", bufs=10) as xtp,
            tc.tile_pool(name="persist", bufs=1) as persist,
            tc.tile_pool(name="ep", bufs=6) as ep,
            tc.tile_pool(name="psA", bufs=2, space="PSUM") as psA,
            tc.tile_pool(name="psB", bufs=2, space="PSUM") as psB,
        ):
            ident = consts.tile([128, 128], bf16, name="ident")
            make_identity(nc, ident)
            wsb, bsb = {}, {}
            for t in "qkv":
                wsb[t] = consts.tile([128, 4, 128], bf16, name=f"w{t}sb", tag=f"w{t}sb")
                nc.sync.dma_start(out=wsb[t], in_=w[t])
                bsb[t] = consts.tile([128, 1], f32, name=f"b{t}sb", tag=f"b{t}sb")
                nc.sync.dma_start(out=bsb[t], in_=bvec[t])
            wosb = consts.tile([128, 4, 128], bf16, name="wosb", tag="wosb")
            nc.sync.dma_start(out=wosb, in_=wo)
            if dve_split:
                # full-width fp32 mantissa mask (Src1 must be a full tensor;
                # [P,1] broadcast Src1 hangs the DVE on this runtime)
                mask_t = consts.tile([128, 384], f32, name="mmask", tag="mmask")
                nc.vector.memset(mask_t.bitcast(i32), 0x007FFFFF)

            NNC = n // 512
            qt_t = [
                persist.tile([128, 512], bf16, name=f"qt{i}", tag=f"qt{i}")
                for i in range(NNC)
            ]
            kt_t = [
                persist.tile([128, 512], bf16, name=f"kt{i}", tag=f"kt{i}")
                for i in range(NNC)
            ]
            vt_t = [
                persist.tile([128, 512], bf16, name=f"vt{i}", tag=f"vt{i}")
                for i in range(NNC)
            ]
            # augmented V chunks: 64 head dims + ones column (col 64)
            vch = [
                [
                    persist.tile(
                        [128, 65], bf16, name=f"vch{h}_{c}", tag=f"vch{h}_{c}"
                    )
                    for c in range(NKC)
                ]
                for h in range(2)
            ]
            # per-qc attention output, both heads packed: rows 64h..64h+63
            ot = [
                persist.tile([128, 512], bf16, name=f"ot{qc}", tag=f"ot{qc}")
                for qc in range(NQC)
            ]
            den_sb = [
                persist.tile([1, n], f32, name=f"den{h}", tag=f"den{h}")
                for h in range(2)
            ]
            for h in range(2):
                for c in range(NKC):
                    nc.vector.memset(vch[h][c][:, 64:65], 1.0)

            # ---- phase 1: projections  t^T = W_p^T @ X^T + b ----
            # nk-outer so early chunks of q/k/v unlock attention ASAP
            dest = {"q": qt_t, "k": kt_t, "v": vt_t}
            for nk in range(NQC):
                for t in "qkv":
                    ppsum = psA.tile([128, 512], f32, name=f"pp_{t}{nk}", tag="s")
                    for dc in range(4):
                        xtile = xtp.tile(
                            [128, 512], bf16, name=f"x_{t}{nk}{dc}", tag="xt"
                        )
                        nc.sync.dma_start(
                            out=xtile,
                            in_=xt[t][dc * 128 : (dc + 1) * 128, nk * 512 : (nk + 1) * 512],
                        )
                        nc.tensor.matmul(
                            ppsum,
                            wsb[t][:, dc, :],
                            xtile,
                            start=(dc == 0),
                            stop=(dc == 3),
                        )
                    if zero_bias:
                        nc.scalar.activation(
                            out=dest[t][nk],
                            in_=ppsum,
                            func=mybir.ActivationFunctionType.Copy,
                        )
                    else:
                        nc.vector.tensor_scalar_add(
                            out=dest[t][nk], in0=ppsum, scalar1=bsb[t]
                        )
                # v^T chunks -> per-head augmented layout
                for c in range(nk * 4, nk * 4 + 4):
                    pt = psA.tile([128, 512], bf16, name=f"pt{c}", tag="s")
                    nc.tensor.transpose(
                        pt[:, 0:128],
                        vt_t[c // 4][:, (c % 4) * 128 : (c % 4 + 1) * 128],
                        ident,
                    )
                    for h in range(2):
                        nc.vector.tensor_copy(
                            out=vch[h][c][:, 0:64], in_=pt[:, h * 64 : (h + 1) * 64]
                        )

            # ---- phase 2: attention; attn@V lags one block so PE never
            # head-of-line blocks on exp ----
            for qc in range(NQC):
                qs = slice(qc * 512, (qc + 1) * 512)
                o_ps = {
                    h: psB.tile([128, 512], f32, name=f"o_{h}_{qc}", tag="oy")
                    for h in range(2)
                }

                def emit_o(blk):
                    k0, blen, e_tiles = blk
                    for h in range(2):
                        for j in range(blen):
                            kc = k0 + j
                            nc.tensor.matmul(
                                o_ps[h][0:65, :],
                                vch[h][kc],
                                e_tiles[h][:, j * 512 : (j + 1) * 512],
                                start=(kc == 0),
                                stop=(kc == NKC - 1),
                                skip_group_check=True,
                            )

                prev = None
                for k0, blen in blocks:
                    e_tiles = {}
                    for h in range(2):
                        hp = slice(h * 64, (h + 1) * 64)
                        s_ps = psA.tile(
                            [128, blen * 512], f32, name=f"s_{h}_{qc}_{k0}", tag="s"
                        )
                        for j in range(blen):
                            kc = k0 + j
                            nc.tensor.matmul(
                                s_ps[:, j * 512 : (j + 1) * 512],
                                kt_t[kc // 4][hp, (kc % 4) * 128 : (kc % 4 + 1) * 128],
                                qt_t[qc][hp, :],
                                start=True,
                                stop=True,
                                skip_group_check=True,
                            )
                        e_sb = ep.tile(
                            [128, blen * 512], bf16, name=f"e_{h}_{qc}_{k0}", tag="e"
                        )
                        u = act_cols(blen)
                        nc.scalar.activation(
                            e_sb[:, 0:u], s_ps[:, 0:u], Exp, scale=0.125
                        )
                        if u < blen * 512:
                            nc.vector.tensor_scalar(
                                out=e_sb[:, u:].bitcast(i16),
                                in0=s_ps[:, u:],
                                scalar1=_A_DVE,
                                scalar2=_B_DVE,
                                op0=mybir.AluOpType.mult,
                                op1=mybir.AluOpType.add,
                            )
                            nc.vector._custom_dve(
                                corr_op,
                                out=e_sb[:, u:],
                                in0=e_sb[:, u:],
                                in1=mask_t[:, 0 : blen * 512 - u],
                                s0=_CORR_A,
                                s1=_CORR_Q2,
                                imm2=_CORR_C,
                            )
                        e_tiles[h] = e_sb
                    if prev is not None:
                        emit_o(prev)
                    prev = (k0, blen, e_tiles)
                emit_o(prev)

                for h in range(2):
                    nc.vector.tensor_copy(
                        out=ot[qc][h * 64 : (h + 1) * 64, :], in_=o_ps[h][0:64, :]
                    )
                    nc.vector.tensor_copy(
                        out=den_sb[h][0:1, qs], in_=o_ps[h][64:65, :]
                    )

            # ---- phase 3: out-projection, heads row-tiled concurrently ----
            for qc in range(NQC):
                qs = slice(qc * 512, (qc + 1) * 512)
                for mt in range(4):
                    for h in range(2):
                        hp = slice(h * 64, (h + 1) * 64)
                        pool, tag = (psA, "s") if h == 0 else (psB, "oy")
                        y_ps = pool.tile(
                            [128, 512], f32, name=f"y_{h}_{qc}_{mt}", tag=tag
                        )
                        nc.tensor.matmul(
                            y_ps,
                            wosb[hp, mt, :],
                            ot[qc][hp, :],
                            start=True,
                            stop=True,
                            skip_group_check=True,
                        )
                        y_sb = xtp.tile(
                            [128, 512], bf16, name=f"ysb_{h}_{qc}_{mt}", tag="ysb"
                        )
                        if h == 0:
                            nc.vector.tensor_copy(out=y_sb, in_=y_ps)
                        else:
                            nc.scalar.copy(out=y_sb, in_=y_ps)
                        nc.sync.dma_start(
                            out=y_out[h][mt * 128 : (mt + 1) * 128, qs], in_=y_sb
                        )
            for h in range(2):
                nc.sync.dma_start(out=den_out[h : h + 1, :], in_=den_sb[h][0:1, :])
    nc.finalize()
    return nc


def get_nc(n=_N, zero_bias=False, dve_split=True):
    key = (n, zero_bias, dve_split)
    if key not in _nc_cache:
        _nc_cache[key] = _build(n, zero_bias, dve_split)
    return _nc_cache[key]


def make_in_maps(Q, K, V, Wq, bq, Wk, bk, Wv, bv, Wo, bo, n=_N):
    """Per-core input dicts (host-side sharding / layout prep)."""
    bf = np.float16
    xts = {}
    for b in range(_B):
        xts[b] = {
            "xtq": np.ascontiguousarray(Q[b][:n].T.astype(bf)),
            "xtk": np.ascontiguousarray(K[b][:n].T.astype(bf)),
            "xtv": np.ascontiguousarray(V[b][:n].T.astype(bf)),
        }
    in_maps = []
    for c in range(_NCORES):
        b, p = divmod(c, 4)
        off = p * 128
        m = dict(xts[b])
        for t, W, bias in (("q", Wq, bq), ("k", Wk, bk), ("v", Wv, bv)):
            m[f"w{t}"] = np.ascontiguousarray(
                W[:, off : off + 128].reshape(4, 128, 128).transpose(1, 0, 2).astype(bf)
            )
            m[f"b{t}"] = np.ascontiguousarray(bias[off : off + 128].reshape(128, 1))
        m["wo"] = np.ascontiguousarray(
            Wo[off : off + 128].reshape(128, 4, 128).astype(bf)
        )
        in_maps.append(m)
    return in_maps


def assemble(results, bo, n=_N):
    """Cross-core reduction: normalize by softmax denominators, sum heads,
    add output bias, restore [B, N, D] layout."""
    out = np.empty((_B, n, _D), np.float32)
    for b in range(_B):
        acc = np.zeros((_D, n), np.float32)
        for p in range(4):
            r = results[4 * b + p]
            for h in range(2):
                acc += r[f"y{h}"].astype(np.float32) / r["den"][h][None, :]
        out[b] = acc.T + bo
    return out


def kernel(Q, K, V, Wq, bq, Wk, bk, Wv, bv, Wo, bo):
    from concourse import bass_utils

    args = [np.asarray(a, np.float32) for a in (Q, K, V, Wq, bq, Wk, bk, Wv, bv, Wo, bo)]
    Q, K, V, Wq, bq, Wk, bk, Wv, bv, Wo, bo = args
    zb = not (np.any(bq) or np.any(bk) or np.any(bv))
    nc = get_nc(zero_bias=zb)
    in_maps = make_in_maps(Q, K, V, Wq, bq, Wk, bk, Wv, bv, Wo, bo)
    res = bass_utils.run_bass_kernel_spmd(
        nc, in_maps, core_ids=list(range(_NCORES))
    )
    return assemble(res.results, bo)


# revision 10
# speedup vs baseline: 1.3984x; 1.1249x over previous
"""Multi-head attention (B=2, N=4096, D=512, H=8) on 8 trn2 NeuronCores.

Sharding: core c handles batch b = c//4 and head-pair p = c%4 (heads 2p,
2p+1).  Each core projects its batch's Q/K/V against its pair's weight
columns, computes transposed attention scores sT = K_h @ Q_h^T, applies
exp((1/sqrt(dk))*sT) split between the ACT engine (table exp, ~75% of
columns) and the DVE (~25%: a Schraudolph int16 bit-trick pass plus a
custom 7-stage correction op that multiplies by a quadratic in the
mantissa, sigma ~0.2%), multiplies by an augmented V (extra ones column,
M=65) so the softmax denominators fall out of the same matmul, and
applies its rows of Wo with both heads row-tiled concurrently in the PE
array.

Schedule notes (all for keeping the PE busy and HAM un-throttled):
  - K/V projections first (interleaved), then per-qc Q projections and
    the out-projection are woven into the attention stream.
  - attn@V matmuls are issued one k-block behind the scores matmuls so
    the PE never head-of-line blocks on exp results.
  - all HBM transfers are whole contiguous [128,512] tiles (host
    pre-tiles), minimizing DMA descriptor overhead.
Normalization by the softmax denominator commutes with the output
projection, so it is applied on the host during the cross-core
reduction.

Device layouts (host pre-arranges):
  xt{q,k,v}: [8, 4, 128, 512]  tile (nk, dc): X^T[dc*128:+128, nk*512:+512]
  w{q,k,v}:  [128, 4, 128]     w[p, dc, c] = W[dc*128+p, off+c]
  b{q,k,v}:  [128, 1]          pair slice of bias
  wo:        [128, 4, 128]     wo[p, mt, c] = Wo[off+p, mt*128+c]
Outputs per core:
  y0, y1: [4, 8, 128, 512]  tile (mt, qc): y^T[mt*128:+128, qc*512:+512]
  den:    [2, N]            softmax denominators per head
Final host step: out[b] = (sum_{p,h} y_h / den_h).T + bo
"""

import numpy as np

_B, _N, _D, _H, _DK = 2, 4096, 512, 8, 64
_NCORES = 8

_LN2 = float(np.log(2.0))
_A_DVE = 0.125 * 1024.0 / _LN2
_B_DVE = 15360.0
_CORR_A = -1.4763417585548537
_CORR_Q2 = 0.22711289921196798
_CORR_C = 0.9424678640725361

_nc_cache = {}
_exp_corr_op = None


def _get_exp_corr_op():
    """Register (once) the custom DVE op: out = ((u+C0)^2*C1 + C2) * Src0
    with u = bitwise_or(bitwise_and(Src0, Src1), 1.0f) — Src1 carries the
    fp32 mantissa mask 0x007FFFFF as a full-width tensor ([P,1] broadcast
    Src1 hangs the DVE on this runtime)."""
    global _exp_corr_op
    if _exp_corr_op is not None:
        return _exp_corr_op
    from concourse import dve_ops
    from concourse.dve_spec import (
        AluOp,
        Bin,
        C0,
        C1,
        C2,
        One,
        Spec,
        Src0,
        Src1,
        lower,
        sq,
    )
    from concourse.dve_uop import DveOpSpec

    name = "EXP16_CORR_ANT"
    for op in dve_ops.OPS:
        if op.name == name:
            _exp_corr_op = op
            return op

    u = Bin(AluOp.BITWISE_OR, Bin(AluOp.BITWISE_AND, Src0, Src1), One)
    body = (sq(u + C0) * C1 + C2) * Src0

    def _ref(in0, in1, s0, s1, imm2):
        b = np.asarray(in0, np.float32).view(np.uint32)
        m = np.asarray(in1, np.float32).view(np.uint32)
        uu = ((b & m) | np.uint32(0x3F800000)).view(np.float32)
        return ((uu + s0) ** 2 * s1 + imm2) * in0

    spec = Spec(body=body, reference=_ref)
    sha = {
        ver: DveOpSpec(name=name, uops=lower(spec, ver=ver)).sha(ver)
        for ver in ("v3", "v4")
    }
    op = dve_ops.DveOp(name, spec, subdim=False, uops_sha=sha)
    idx = len(dve_ops.OPS)
    dve_ops.OPS.append(op)
    dve_ops.CUSTOM_DVE_SPECS[name] = spec
    dve_ops._SUB_OPCODE_FOR_NAME[name] = dve_ops._CUSTOM_DVE_ROW_BASE + idx
    _exp_corr_op = op
    return op


def _build(n=_N, zero_bias=False, dve_split=True):
    import concourse.mybir as mybir
    import concourse.tile as tile
    from concourse import bacc
    from concourse.masks import make_identity

    f32 = mybir.dt.float32
    i16 = mybir.dt.int16
    i32 = mybir.dt.int32
    bf16 = mybir.dt.float16
    Exp = mybir.ActivationFunctionType.Exp
    NKC = n // 128  # k chunks of 128 (columns of sT)
    NQC = n // 512  # q chunks of 512
    BL = 2  # k-chunks per exp block
    blocks = []
    i = 0
    while i < NKC:
        blen = min(BL, NKC - i)
        blocks.append((i, blen))
        i += blen

    corr_op = _get_exp_corr_op() if dve_split else None

    def act_cols(blen):
        if not dve_split:
            return blen * 512
        return (int(blen * 512 * 0.75) // 32) * 32

    nc = bacc.Bacc(
        "TRN2", target_bir_lowering=False, debug=False, num_devices=_NCORES
    )

    xt = {
        t: nc.dram_tensor(f"xt{t}", [NQC, 4, 128, 512], bf16, kind="ExternalInput").ap()
        for t in "qkv"
    }
    w = {
        t: nc.dram_tensor(f"w{t}", [128, 4, 128], bf16, kind="ExternalInput").ap()
        for t in "qkv"
    }
    bvec = {
        t: nc.dram_tensor(f"b{t}", [128, 1], f32, kind="ExternalInput").ap()
        for t in "qkv"
    }
    wo = nc.dram_tensor("wo", [128, 4, 128], bf16, kind="ExternalInput").ap()
    y_out = [
        nc.dram_tensor(f"y{h}", [4, NQC, 128, 512], bf16, kind="ExternalOutput").ap()
        for h in range(2)
    ]
    den_out = nc.dram_tensor("den", [2, n], f32, kind="ExternalOutput").ap()

    with tile.TileContext(nc) as tc:
        with (
            tc.tile_pool(name="consts", bufs=1) as consts,
            tc.tile_pool(name="xtp", bufs=10) as xtp,
            tc.tile_pool(name="persist", bufs=1) as persist,
            tc.tile_pool(name="ep", bufs=6) as ep,
            tc.tile_pool(name="psA", bufs=2, space="PSUM") as psA,
            tc.tile_pool(name="psB", bufs=2, space="PSUM") as psB,
            tc.tile_pool(name="psC", bufs=2, space="PSUM") as psC,
        ):
            ident = consts.tile([128, 128], bf16, name="ident")
            make_identity(nc, ident)
            wsb, bsb = {}, {}
            for t in "qkv":
                wsb[t] = consts.tile([128, 4, 128], bf16, name=f"w{t}sb", tag=f"w{t}sb")
                nc.sync.dma_start(out=wsb[t], in_=w[t])
                bsb[t] = consts.tile([128, 1], f32, name=f"b{t}sb", tag=f"b{t}sb")
                nc.sync.dma_start(out=bsb[t], in_=bvec[t])
            wosb = consts.tile([128, 4, 128], bf16, name="wosb", tag="wosb")
            nc.sync.dma_start(out=wosb, in_=wo)
            if dve_split:
                mask_t = consts.tile([128, 256], f32, name="mmask", tag="mmask")
                nc.vector.memset(mask_t.bitcast(i32), 0x007FFFFF)

            qt_t = [
                persist.tile([128, 512], bf16, name=f"qt{i}", tag=f"qt{i}")
                for i in range(NQC)
            ]
            kt_t = [
                persist.tile([128, 512], bf16, name=f"kt{i}", tag=f"kt{i}")
                for i in range(NQC)
            ]
            vt_t = [
                persist.tile([128, 512], bf16, name=f"vt{i}", tag=f"vt{i}")
                for i in range(NQC)
            ]
            # augmented V chunks: 64 head dims + ones column (col 64)
            vch = [
                [
                    persist.tile(
                        [128, 65], bf16, name=f"vch{h}_{c}", tag=f"vch{h}_{c}"
                    )
                    for c in range(NKC)
                ]
                for h in range(2)
            ]
            ot = [
                persist.tile([128, 512], bf16, name=f"ot{qc}", tag=f"ot{qc}")
                for qc in range(NQC)
            ]
            den_sb = [
                persist.tile([1, n], f32, name=f"den{h}", tag=f"den{h}")
                for h in range(2)
            ]
            for h in range(2):
                for c in range(NKC):
                    nc.vector.memset(vch[h][c][:, 64:65], 1.0)

            def proj(t, nk, dst):
                ppsum = psC.tile([128, 512], f32, name=f"pp_{t}{nk}", tag="y")
                for dc in range(4):
                    xtile = xtp.tile(
                        [128, 512], bf16, name=f"x_{t}{nk}{dc}", tag="xt"
                    )
                    nc.sync.dma_start(out=xtile, in_=xt[t][nk, dc])
                    nc.tensor.matmul(
                        ppsum,
                        wsb[t][:, dc, :],
                        xtile,
                        start=(dc == 0),
                        stop=(dc == 3),
                    )
                if zero_bias:
                    nc.scalar.activation(
                        out=dst, in_=ppsum, func=mybir.ActivationFunctionType.Copy
                    )
                else:
                    nc.vector.tensor_scalar_add(out=dst, in0=ppsum, scalar1=bsb[t])

            # ---- phase 1: K/V projections (interleaved); Q projections are
            # woven into the attention stream with one-qc lookahead ----
            for nk in range(NQC):
                proj("k", nk, kt_t[nk])
                proj("v", nk, vt_t[nk])
                for c in range(nk * 4, nk * 4 + 4):
                    pt = psC.tile([128, 512], bf16, name=f"pt{c}", tag="y")
                    nc.tensor.transpose(
                        pt[:, 0:128],
                        vt_t[c // 4][:, (c % 4) * 128 : (c % 4 + 1) * 128],
                        ident,
                    )
                    for h in range(2):
                        nc.vector.tensor_copy(
                            out=vch[h][c][:, 0:64], in_=pt[:, h * 64 : (h + 1) * 64]
                        )
            proj("q", 0, qt_t[0])
            proj("q", 1, qt_t[1])

            # ---- phase 2: attention + woven out-projection ----
            def outproj(qc):
                qs = slice(qc * 512, (qc + 1) * 512)
                for mt in range(4):
                    for h in range(2):
                        hp = slice(h * 64, (h + 1) * 64)
                        y_ps = psC.tile(
                            [128, 512], f32, name=f"y_{h}_{qc}_{mt}", tag="y"
                        )
                        nc.tensor.matmul(
                            y_ps,
                            wosb[hp, mt, :],
                            ot[qc][hp, :],
                            start=True,
                            stop=True,
                            skip_group_check=True,
                        )
                        y_sb = xtp.tile(
                            [128, 512], bf16, name=f"ysb_{h}_{qc}_{mt}", tag="ysb"
                        )
                        if h == 0:
                            nc.vector.tensor_copy(out=y_sb, in_=y_ps)
                        else:
                            nc.scalar.copy(out=y_sb, in_=y_ps)
                        nc.sync.dma_start(out=y_out[h][mt, qc], in_=y_sb)

            for qc in range(NQC):
                qs = slice(qc * 512, (qc + 1) * 512)
                o_ps = {
                    h: psB.tile([128, 512], f32, name=f"o_{h}_{qc}", tag="oy")
                    for h in range(2)
                }

                def emit_o(blk):
                    k0, blen, e_tiles = blk
                    for h in range(2):
                        for j in range(blen):
                            kc = k0 + j
                            nc.tensor.matmul(
                                o_ps[h][0:65, :],
                                vch[h][kc],
                                e_tiles[h][:, j * 512 : (j + 1) * 512],
                                start=(kc == 0),
                                stop=(kc == NKC - 1),
                                skip_group_check=True,
                            )

                prev = None
                for k0, blen in blocks:
                    e_tiles = {}
                    for h in range(2):
                        hp = slice(h * 64, (h + 1) * 64)
                        s_ps = psA.tile(
                            [128, blen * 512], f32, name=f"s_{h}_{qc}_{k0}", tag="s"
                        )
                        for j in range(blen):
                            kc = k0 + j
                            nc.tensor.matmul(
                                s_ps[:, j * 512 : (j + 1) * 512],
                                kt_t[kc // 4][hp, (kc % 4) * 128 : (kc % 4 + 1) * 128],
                                qt_t[qc][hp, :],
                                start=True,
                                stop=True,
                                skip_group_check=True,
                            )
                        e_sb = ep.tile(
                            [128, blen * 512], bf16, name=f"e_{h}_{qc}_{k0}", tag="e"
                        )
                        u = act_cols(blen)
                        nc.scalar.activation(
                            e_sb[:, 0:u], s_ps[:, 0:u], Exp, scale=0.125
                        )
                        if u < blen * 512:
                            nc.vector.tensor_scalar(
                                out=e_sb[:, u:].bitcast(i16),
                                in0=s_ps[:, u:],
                                scalar1=_A_DVE,
                                scalar2=_B_DVE,
                                op0=mybir.AluOpType.mult,
                                op1=mybir.AluOpType.add,
                            )
                            nc.vector._custom_dve(
                                corr_op,
                                out=e_sb[:, u:],
                                in0=e_sb[:, u:],
                                in1=mask_t[:, 0 : blen * 512 - u],
                                s0=_CORR_A,
                                s1=_CORR_Q2,
                                imm2=_CORR_C,
                            )
                        e_tiles[h] = e_sb
                    if prev is not None:
                        emit_o(prev)
                    prev = (k0, blen, e_tiles)
                emit_o(prev)

                for h in range(2):
                    nc.vector.tensor_copy(
                        out=ot[qc][h * 64 : (h + 1) * 64, :], in_=o_ps[h][0:64, :]
                    )
                    nc.vector.tensor_copy(
                        out=den_sb[h][0:1, qs], in_=o_ps[h][64:65, :]
                    )
                if qc + 2 < NQC:
                    proj("q", qc + 2, qt_t[qc + 2])
                outproj(qc)

            for h in range(2):
                nc.sync.dma_start(out=den_out[h : h + 1, :], in_=den_sb[h][0:1, :])
    nc.finalize()
    return nc


def get_nc(n=_N, zero_bias=False, dve_split=True):
    key = (n, zero_bias, dve_split)
    if key not in _nc_cache:
        _nc_cache[key] = _build(n, zero_bias, dve_split)
    return _nc_cache[key]


def make_in_maps(Q, K, V, Wq, bq, Wk, bk, Wv, bv, Wo, bo, n=_N):
    """Per-core input dicts (host-side sharding / layout prep)."""
    bf = np.float16
    nqc = n // 512
    xts = {}
    for b in range(_B):
        d = {}
        for t, X in (("q", Q), ("k", K), ("v", V)):
            xt = X[b][:n].T.astype(bf)  # [512, n]
            d[f"xt{t}"] = np.ascontiguousarray(
                xt.reshape(4, 128, nqc, 512).transpose(2, 0, 1, 3)
            )
        xts[b] = d
    in_maps = []
    for c in range(_NCORES):
        b, p = divmod(c, 4)
        off = p * 128
        m = dict(xts[b])
        for t, W, bias in (("q", Wq, bq), ("k", Wk, bk), ("v", Wv, bv)):
            m[f"w{t}"] = np.ascontiguousarray(
                W[:, off : off + 128].reshape(4, 128, 128).transpose(1, 0, 2).astype(bf)
            )
            m[f"b{t}"] = np.ascontiguousarray(bias[off : off + 128].reshape(128, 1))
        m["wo"] = np.ascontiguousarray(
            Wo[off : off + 128].reshape(128, 4, 128).astype(bf)
        )
        in_maps.append(m)
    return in_maps


def assemble(results, bo, n=_N):
    """Cross-core reduction: normalize by softmax denominators, sum heads,
    add output bias, restore [B, N, D] layout."""
    nqc = n // 512
    out = np.empty((_B, n, _D), np.float32)
    for b in range(_B):
        acc = np.zeros((_D, n), np.float32)
        for p in range(4):
            r = results[4 * b + p]
            for h in range(2):
                # y [4, nqc, 128, 512] -> [512, n]
                y = (
                    r[f"y{h}"]
                    .astype(np.float32)
                    .transpose(0, 2, 1, 3)
                    .reshape(_D, n)
                )
                acc += y / r["den"][h][None, :]
        out[b] = acc.T + bo
    return out


def kernel(Q, K, V, Wq, bq, Wk, bk, Wv, bv, Wo, bo):
    from concourse import bass_utils

    args = [np.asarray(a, np.float32) for a in (Q, K, V, Wq, bq, Wk, bk, Wv, bv, Wo, bo)]
    Q, K, V, Wq, bq, Wk, bk, Wv, bv, Wo, bo = args
    zb = not (np.any(bq) or np.any(bk) or np.any(bv))
    nc = get_nc(zero_bias=zb)
    in_maps = make_in_maps(Q, K, V, Wq, bq, Wk, bk, Wv, bv, Wo, bo)
    res = bass_utils.run_bass_kernel_spmd(
        nc, in_maps, core_ids=list(range(_NCORES))
    )
    return assemble(res.results, bo)
